# revision 34
# baseline (speedup 1.0000x reference)
"""Trainium2 Bass kernel for nn_AggrHGraphConvWindow (3x GraphConv -> LeakyReLU -> 2-layer LSTM).

Contract: kernel(**inputs) takes FULL unsharded numpy inputs, returns FULL output
(33500, 16, 128) float32.  Internally shards destination rows across 8 NeuronCores
(graph/data parallel per the sharding hint: edges partitioned by destination with
halo exchange of source features), runs one SPMD Bass program, and gathers.
"""

import os
import numpy as np
import ml_dtypes

BF16 = np.float16  # fp16: same cost as bf16 on PE/DVE, 8x finer mantissa

# Problem constants (hardcoded per spec)
N_NODE, N_POD, N_SVC = 500, 30000, 3000
T, F, H = 16, 64, 128
NCORES = 8
P = 128

NODE_PC = 64     # nodes per core (64*8=512 >= 500)
POD_PC = 3750    # pods per core (exact)
SVC_PC = 376     # svcs per core (376*8=3008 >= 3000)

NODE_TILES = 1   # 64 real rows inside one 128-row tile
POD_TILES = (POD_PC + P - 1) // P   # 30
SVC_TILES = (SVC_PC + P - 1) // P   # 3
N_TILES = NODE_TILES + POD_TILES + SVC_TILES  # 34
R_CORE = N_TILES * P  # 4352 rows per core (padded)

# LSTM batch tiles over the 4352 local rows
LSTM_TILES = [(j * 512, 512) for j in range(R_CORE // 512)]
if R_CORE % 512:
    LSTM_TILES.append((512 * (R_CORE // 512), R_CORE % 512))

_COMPILED = {}


# ----------------------------------------------------------------------------
# Host-side preprocessing: edge routing, degree norms, halo tables, weight prep
# ----------------------------------------------------------------------------

def _degrees(src, dst, n_src, n_dst):
    dout = np.bincount(src, minlength=n_src).astype(np.float64)
    din = np.bincount(dst, minlength=n_dst).astype(np.float64)
    return (1.0 / np.sqrt(np.maximum(dout, 1.0)), 1.0 / np.sqrt(np.maximum(din, 1.0)))


def _prep(inputs):
    nf = np.asarray(inputs["node_feat"]).reshape(N_NODE, T * F)
    pf = np.asarray(inputs["pod_feat"]).reshape(N_POD, T * F)
    sf = np.asarray(inputs["svc_feat"]).reshape(N_SVC, T * F)

    in_src = np.asarray(inputs["inst_node_src"]).astype(np.int64)
    in_dst = np.asarray(inputs["inst_node_dst"]).astype(np.int64)
    ni_src = np.asarray(inputs["node_inst_src"]).astype(np.int64)
    ni_dst = np.asarray(inputs["node_inst_dst"]).astype(np.int64)
    sc_src = np.asarray(inputs["svc_call_src"]).astype(np.int64)
    sc_dst = np.asarray(inputs["svc_call_dst"]).astype(np.int64)

    # normalization: x/sqrt(deg_out) -> segsum -> /sqrt(deg_in), folded per-edge
    ro_in, ri_in = _degrees(in_src, in_dst, N_POD, N_NODE)
    ro_ni, ri_ni = _degrees(ni_src, ni_dst, N_NODE, N_POD)
    ro_sc, ri_sc = _degrees(sc_src, sc_dst, N_SVC, N_SVC)

    # Route edges: per (core, tile) buckets.
    # tile order within core: pods tiles 0..29, svc 30..32, node 33 (node last)
    def route(src, dst, w, kind):
        if kind == 0:    # dst = node -> last tile (heaviest; keeps LSTM ramp fast)
            core = dst // NODE_PC
            q = dst - core * NODE_PC
            tile = np.full_like(dst, POD_TILES + SVC_TILES)
            row = q
        elif kind == 1:  # dst = pod -> tiles [0, POD_TILES)
            core = dst // POD_PC
            q = dst - core * POD_PC
            tile = q // P
            row = q % P
        else:            # dst = svc -> tiles [POD_TILES, POD_TILES+SVC_TILES)
            core = dst // SVC_PC
            q = dst - core * SVC_PC
            tile = POD_TILES + q // P
            row = q % P
        return core, tile, row, src, w

    ew_in = (ro_in[in_src] * ri_in[in_dst]).astype(np.float32)
    ew_ni = (ro_ni[ni_src] * ri_ni[ni_dst]).astype(np.float32)
    ew_sc = (ro_sc[sc_src] * ri_sc[sc_dst]).astype(np.float32)

    routed = {
        0: route(in_src, in_dst, ew_in, 0),   # node phase: src = pods
        1: route(ni_src, ni_dst, ew_ni, 1),   # pod phase:  src = nodes
        2: route(sc_src, sc_dst, ew_sc, 2),   # svc phase:  src = svcs
    }

    # per (core, tile) edge lists
    buckets = [[([], [], []) for _ in range(N_TILES)] for _ in range(NCORES)]
    for kind in (0, 1, 2):
        core, tile, row, src, w = routed[kind]
        order = np.lexsort((row, tile, core))
        core, tile, row, src, w = core[order], tile[order], row[order], src[order], w[order]
        # group
        key = core * N_TILES + tile
        uniq, starts = np.unique(key, return_index=True)
        starts = list(starts) + [len(key)]
        for ui, k in enumerate(uniq):
            c, t = int(k) // N_TILES, int(k) % N_TILES
            s, e = starts[ui], starts[ui + 1]
            buckets[c][t] = (src[s:e], row[s:e], w[s:e])

    # static chunk counts per tile (max over cores), >= 1
    K = []
    for t in range(N_TILES):
        mx = 1
        for c in range(NCORES):
            mx = max(mx, (len(buckets[c][t][0]) + P - 1) // P)
        K.append(mx)
    base = np.concatenate([[0], np.cumsum(K)]).astype(np.int64)
    C_total = int(base[-1])

    # halo source tables (bf16) + remapped edge arrays
    PN = K[-1] * P  # node-phase table rows (pods referenced by this core's edges)
    PS = sum(K[POD_TILES:POD_TILES + SVC_TILES]) * P  # svc-phase table rows
    nodetab = np.zeros((512, T * F), dtype=BF16)
    nodetab[:N_NODE] = nf.astype(BF16)

    in_maps = []
    for c in range(NCORES):
        esrc = np.zeros((C_total, P), dtype=np.int32)
        edst = np.zeros((C_total, P), dtype=np.float32)
        ew = np.zeros((C_total, P), dtype=np.float32)

        # node phase: compact pod rows
        NODE_TILE = POD_TILES + SVC_TILES
        psrc, prow, pw = buckets[c][NODE_TILE]
        uniq, inv = np.unique(psrc, return_inverse=True) if len(psrc) else (np.zeros(0, np.int64), np.zeros(0, np.int64))
        podtab = np.zeros((PN, T * F), dtype=BF16)
        podtab[:len(uniq)] = pf[uniq].astype(BF16)

        svctab = np.zeros((PS, T * F), dtype=BF16)
        svc_fill = 0

        for t in range(N_TILES):
            src, row, w = buckets[c][t]
            n = len(src)
            if t == NODE_TILE:
                sidx = inv
            elif t < POD_TILES:
                sidx = src  # direct index into nodetab
            else:
                u2, i2 = np.unique(src, return_inverse=True) if n else (np.zeros(0, np.int64), np.zeros(0, np.int64))
                svctab[svc_fill:svc_fill + len(u2)] = sf[u2].astype(BF16)
                sidx = i2 + svc_fill
                svc_fill += len(u2)
            b0 = int(base[t]) * P
            esrc.reshape(-1)[b0:b0 + n] = sidx
            edst.reshape(-1)[b0:b0 + n] = row
            ew.reshape(-1)[b0:b0 + n] = w

        m = {
            "podtab": podtab, "nodetab": nodetab, "svctab": svctab,
            "esrc": np.ascontiguousarray(esrc.T),
            "esrcf": np.ascontiguousarray(esrc.T.astype(np.float32)),
            "edst": np.ascontiguousarray(edst.T),
            "ew": np.ascontiguousarray(ew.T),
        }
        in_maps.append(m)

    # ---- weights (identical on all cores) ----
    def conv_w(Wname):
        W = np.asarray(inputs[Wname])  # (T, F, H)
        wt = W.transpose(1, 0, 2).reshape(F, T * H)  # (64, 2048) F-major
        return np.vstack([wt, wt]).astype(BF16)       # (128, 2048) vertical dup

    def conv_b(bname):
        return np.asarray(inputs[bname]).reshape(1, T * H).astype(BF16)

    def lstm_w(Wname):
        # rows [i,f,g,o] -> [i,f,o,g]; g block doubled so tanh(g) = 2*sigmoid(2g)-1
        # lets one Sigmoid cover all four gate chunks.
        W = np.asarray(inputs[Wname])  # (512, in_dim)
        Wp = np.concatenate([W[0:128], W[128:256], W[384:512], 2.0 * W[256:384]], axis=0)
        return np.ascontiguousarray(Wp.T).astype(BF16)  # (in_dim, 512), [i,f,o,2g]

    def lstm_b(b1, b2):
        b = np.asarray(inputs[b1]) + np.asarray(inputs[b2])
        bp = np.concatenate([b[0:128], b[128:256], b[384:512], 2.0 * b[256:384]])
        return bp.reshape(1, 512).astype(BF16)

    shared = {
        "wt_node": conv_w("W_in"), "wt_pod": conv_w("W_ni"), "wt_svc": conv_w("W_svc"),
        "bt_node": conv_b("b_in"), "bt_pod": conv_b("b_ni"), "bt_svc": conv_b("b_svc"),
        "wih0": lstm_w("Wih0"), "whh0": lstm_w("Whh0"),
        "wih1": lstm_w("Wih1"), "whh1": lstm_w("Whh1"),
        "bias0": lstm_b("bih0", "bhh0"), "bias1": lstm_b("bih1", "bhh1"),
        "iota": np.broadcast_to(np.arange(P, dtype=np.float32), (P, P)).copy(),
        "iota512": np.broadcast_to(np.arange(512, dtype=np.float32), (P, 512)).copy(),
    }
    for m in in_maps:
        m.update(shared)

    meta = (C_total, tuple(K), PN, PS)
    return meta, in_maps


# ----------------------------------------------------------------------------
# Device program
# ----------------------------------------------------------------------------

def _build(meta):
    import concourse.bass as bass
    import concourse.tile as tile
    import concourse.mybir as mybir

    C_total, K, PN, PS = meta
    f32 = mybir.dt.float32
    bf16 = mybir.dt.float16
    i32 = mybir.dt.int32
    AF = mybir.ActivationFunctionType
    ALU = mybir.AluOpType

    import concourse.bacc as bacc
    nc = bacc.Bacc("TRN2", target_bir_lowering=False, debug=False, enable_asserts=False)

    podtab = nc.dram_tensor("podtab", [PN, T * F], bf16, kind="ExternalInput")
    nodetab = nc.dram_tensor("nodetab", [512, T * F], bf16, kind="ExternalInput")
    svctab = nc.dram_tensor("svctab", [PS, T * F], bf16, kind="ExternalInput")
    esrc_d = nc.dram_tensor("esrc", [P, C_total], i32, kind="ExternalInput")
    esrcf_d = nc.dram_tensor("esrcf", [P, C_total], f32, kind="ExternalInput")
    edst_d = nc.dram_tensor("edst", [P, C_total], f32, kind="ExternalInput")
    ew_d = nc.dram_tensor("ew", [P, C_total], f32, kind="ExternalInput")
    wt_d = {k: nc.dram_tensor(f"wt_{k}", [P, T * H], bf16, kind="ExternalInput")
            for k in ("node", "pod", "svc")}
    bt_d = {k: nc.dram_tensor(f"bt_{k}", [1, T * H], bf16, kind="ExternalInput")
            for k in ("node", "pod", "svc")}
    wih_d = [nc.dram_tensor(f"wih{l}", [H, 512], bf16, kind="ExternalInput") for l in range(2)]
    whh_d = [nc.dram_tensor(f"whh{l}", [H, 512], bf16, kind="ExternalInput") for l in range(2)]
    bias_d = [nc.dram_tensor(f"bias{l}", [1, 512], bf16, kind="ExternalInput") for l in range(2)]
    iota_d = nc.dram_tensor("iota", [P, P], f32, kind="ExternalInput")
    iota512_d = nc.dram_tensor("iota512", [P, 512], f32, kind="ExternalInput")
    out_d = nc.dram_tensor("out", [P, T * R_CORE], bf16, kind="ExternalOutput")

    tile_kind = (["pod"] * POD_TILES + ["svc"] * SVC_TILES + ["node"] * NODE_TILES)
    base = np.concatenate([[0], np.cumsum(K)]).astype(int)

    NJ = len(LSTM_TILES)
    with tile.TileContext(nc) as tc:
        with tc.tile_pool(name="dram", bufs=NJ, space="DRAM") as dramp, \
             tc.tile_pool(name="const", bufs=1) as constp:
            # x0 spill split per LSTM batch tile so the LSTM can start on tile j
            # as soon as its 4 conv row-tiles are written (pipeline the phases)
            x0p = [dramp.tile([P, T * B], bf16, tag="x0p", name=f"x0p_{j}")
                   for j, (r0, B) in enumerate(LSTM_TILES)]

            # load constants
            esrc_sb = constp.tile([P, C_total], i32)
            esrcf_sb = constp.tile([P, C_total], f32)
            edst_sb = constp.tile([P, C_total], f32)
            ew_sb = constp.tile([P, C_total], f32)
            iota_sb = constp.tile([P, P], f32)
            iota512_sb = constp.tile([P, 512], f32)
            nc.sync.dma_start(esrc_sb[:], esrc_d.ap())
            nc.sync.dma_start(esrcf_sb[:], esrcf_d.ap())
            nc.sync.dma_start(edst_sb[:], edst_d.ap())
            nc.sync.dma_start(ew_sb[:], ew_d.ap())
            nc.sync.dma_start(iota_sb[:], iota_d.ap())
            nc.sync.dma_start(iota512_sb[:], iota512_d.ap())
            # node feature table resident in SBUF for the pod-phase one-hot
            # matmul conv (replaces 15MB of per-edge row gathers)
            ntab_sb = []
            for sc in range(4):
                ntab_sb.append(constp.tile([P, T * F], bf16, name=f"ntab_{sc}"))
                nc.sync.dma_start(ntab_sb[sc][:], nodetab.ap()[sc * P:(sc + 1) * P, :])
            wt_sb, bt_sb = {}, {}
            for k in ("node", "pod", "svc"):
                wt_sb[k] = constp.tile([P, T * H], bf16, name=f"wt_{k}_sb")
                bt_sb[k] = constp.tile([1, T * H], bf16, name=f"bt_{k}_sb")
                nc.sync.dma_start(wt_sb[k][:], wt_d[k].ap())
                nc.sync.dma_start(bt_sb[k][:], bt_d[k].ap())
            wih_sb, whh_sb, bias_sb = [], [], []
            for l in range(2):
                wih_sb.append(constp.tile([H, 512], bf16, name=f"wih{l}_sb"))
                whh_sb.append(constp.tile([H, 512], bf16, name=f"whh{l}_sb"))
                bias_sb.append(constp.tile([1, 512], bf16, name=f"bias{l}_sb"))
                nc.sync.dma_start(wih_sb[l][:], wih_d[l].ap())
                nc.sync.dma_start(whh_sb[l][:], whh_d[l].ap())
                nc.sync.dma_start(bias_sb[l][:], bias_d[l].ap())
            ones_sb = constp.tile([1, 512], bf16)
            nc.gpsimd.memset(ones_sb[:], 1.0)

            srctabs = {"node": podtab, "pod": nodetab, "svc": svctab}

            # Conv + LSTM share one scope (and one PSUM pool) so the two
            # phases pipeline: LSTM batch-tile j starts once its 4 conv
            # row-tiles have spilled.
            with tc.tile_pool(name="gat", bufs=6) as gatp, \
                 tc.tile_pool(name="ssb", bufs=6) as ssbp, \
                 tc.tile_pool(name="psum", bufs=2, space="PSUM") as psump, \
                 tc.tile_pool(name="aggsb", bufs=3) as aggsbp, \
                 tc.tile_pool(name="x0sb", bufs=3) as x0sbp, \
                 tc.tile_pool(name="st_h0", bufs=NJ) as ph0, \
                 tc.tile_pool(name="st_c0", bufs=NJ) as pc0, \
                 tc.tile_pool(name="st_h1", bufs=NJ) as ph1, \
                 tc.tile_pool(name="st_c1", bufs=NJ) as pc1, \
                 tc.tile_pool(name="ifo", bufs=4) as ifop, \
                 tc.tile_pool(name="gt", bufs=4) as gtp, \
                 tc.tile_pool(name="tcl", bufs=4) as tclp, \
                 tc.tile_pool(name="tmp1", bufs=2) as t1p, \
                 tc.tile_pool(name="tmp2", bufs=2) as t2p, \
                 tc.tile_pool(name="xin", bufs=6) as xinp, \
                 tc.tile_pool(name="yout", bufs=3) as youtp:

                def conv_group(d, k0, k1, gs, ss, agg):
                    # PE-accumulate chunk group [k0,k1] into one PSUM partial
                    # (m-outer so per-bank psum accumulation groups stay
                    # sequential), then DVE-combine into agg.
                    pp = psump.tile([P, T * H], f32, tag="ps", name=f"pp_{d}_{k0}")
                    for m in range(8):
                        for kk in range(k0, k1 + 1):
                            nc.tensor.matmul(
                                out=pp[:, m * P:(m + 1) * P],
                                lhsT=gs[kk][:, m * P:(m + 1) * P],
                                rhs=ss[kk][:], start=(kk == k0), stop=(kk == k1))
                    if k0 == 0:
                        nc.vector.tensor_copy(agg[:], pp[:, 0:T * F])
                    else:
                        nc.vector.tensor_tensor(out=agg[:], in0=agg[:],
                                                in1=pp[:, 0:T * F], op=ALU.add)

                def conv_tail(d, kind, agg):
                    # linear per timestep + bias (rank-1 matmul) -> PSUM,
                    # then LeakyReLU(0.01) -> bf16 -> spill
                    hx = psump.tile([P, T * H], f32, tag="ps", name=f"hx_{d}")
                    wt, bt = wt_sb[kind], bt_sb[kind]
                    for t in range(T):
                        pb = 64 * (t % 2)
                        nc.tensor.matmul(
                            out=hx[:, t * H:(t + 1) * H], lhsT=bt[0:1, t * H:(t + 1) * H],
                            rhs=ones_sb[0:1, 0:P], start=True, stop=False)
                        nc.tensor.matmul(
                            out=hx[:, t * H:(t + 1) * H],
                            lhsT=wt[pb:pb + F, t * H:(t + 1) * H],
                            rhs=agg[pb:pb + F, (t // 2) * P:(t // 2 + 1) * P],
                            start=False, stop=True)
                    x0t = x0sbp.tile([P, T * H], bf16, tag="x0", name=f"x0t_{d}")
                    nc.scalar.activation(x0t[:], hx[:], AF.Prelu, alpha=0.01)
                    j = min(d // 4, NJ - 1)
                    rl = P * (d - 4 * j)
                    nc.sync.dma_start(
                        x0p[j][:].rearrange("h (t r) -> h t r", t=T)[:, :, rl:rl + P],
                        x0t[:].rearrange("h (t r) -> h t r", t=T))

                def conv_tile_pod(d):
                    # Pod conv via one-hot matmuls against the SBUF-resident
                    # node table: C[s,r] = sum_e w_e [src=s][dst=r], then
                    # agg.T = nodetab.T @ C. No DRAM gather at all.
                    Kd = K[d]
                    agg = aggsbp.tile([P, T * F], bf16, tag="agg", name=f"agg_{d}")
                    ohs, s0s = [], []
                    for ki in range(Kd):
                        col = int(base[d]) + ki
                        oh = gatp.tile([P, 512], bf16, tag="g", name=f"oh_{d}_{ki}")
                        nc.vector.tensor_scalar(
                            out=oh[:], in0=iota512_sb[:],
                            scalar1=esrcf_sb[:, col:col + 1], scalar2=ew_sb[:, col:col + 1],
                            op0=ALU.is_equal, op1=ALU.mult)
                        s0 = ssbp.tile([P, P], bf16, tag="s", name=f"s0_{d}_{ki}")
                        nc.vector.tensor_scalar(
                            out=s0[:], in0=iota_sb[:],
                            scalar1=edst_sb[:, col:col + 1], scalar2=None,
                            op0=ALU.is_equal)
                        ohs.append(oh)
                        s0s.append(s0)
                    cps = psump.tile([P, T * H], f32, tag="ps", name=f"cps_{d}")
                    for sc in range(4):
                        for ki in range(Kd):
                            nc.tensor.matmul(
                                out=cps[:, sc * P:(sc + 1) * P],
                                lhsT=ohs[ki][:, sc * P:(sc + 1) * P], rhs=s0s[ki][:],
                                start=(ki == 0), stop=(ki == Kd - 1))
                    csb = ssbp.tile([P, 512], bf16, tag="csb", name=f"csb_{d}")
                    nc.vector.tensor_copy(csb[:], cps[:, 0:512])
                    # reuse the same PSUM tile for agg (start=True overwrites);
                    # keeps pod tiles at 2 psum allocations for 2-slot pipelining
                    for m in range(8):
                        for sc in range(4):
                            nc.tensor.matmul(
                                out=cps[:, m * P:(m + 1) * P],
                                lhsT=ntab_sb[sc][:, m * P:(m + 1) * P],
                                rhs=csb[:, sc * P:(sc + 1) * P],
                                start=(sc == 0), stop=(sc == 3))
                    nc.vector.tensor_copy(agg[:], cps[:, 0:T * F])
                    conv_tail(d, "pod", agg)

                def conv_tile(d):
                    kind = tile_kind[d]
                    Kd = K[d]
                    tab = srctabs[kind]
                    agg = aggsbp.tile([P, T * F], bf16, tag="agg", name=f"agg_{d}")
                    gs, ss = [], []
                    for ki in range(Kd):
                        col = int(base[d]) + ki
                        g = gatp.tile([P, T * F], bf16, tag="g", name=f"g_{d}_{ki}")
                        nc.gpsimd.indirect_dma_start(
                            out=g[:], out_offset=None, in_=tab.ap(),
                            in_offset=bass.IndirectOffsetOnAxis(ap=esrc_sb[:, col:col + 1], axis=0))
                        s = ssbp.tile([P, P], bf16, tag="s", name=f"s_{d}_{ki}")
                        nc.vector.tensor_scalar(
                            out=s[:], in0=iota_sb[:],
                            scalar1=edst_sb[:, col:col + 1], scalar2=ew_sb[:, col:col + 1],
                            op0=ALU.is_equal, op1=ALU.mult)
                        gs.append(g)
                        ss.append(s)
                        if ki % 4 == 3 or ki == Kd - 1:
                            conv_group(d, (ki // 4) * 4, ki, gs, ss, agg)
                    conv_tail(d, kind, agg)

                # ---------------- LSTM phase ----------------
                gatesp = psump

                h = [[None] * NJ, [None] * NJ]
                c = [[None] * NJ, [None] * NJ]
                for j, (r0, B) in enumerate(LSTM_TILES):
                    h[0][j] = ph0.tile([P, B], bf16, tag="h0", name=f"h0_{j}")
                    c[0][j] = pc0.tile([P, B], f32, tag="c0", name=f"c0_{j}")
                    h[1][j] = ph1.tile([P, B], bf16, tag="h1", name=f"h1_{j}")
                    c[1][j] = pc1.tile([P, B], f32, tag="c1", name=f"c1_{j}")

                outr = out_d.ap().rearrange("h (t r) -> h t r", t=T)

                def cell(l, j, t, xin_l, B):
                    # gates [i,f,o,2g]; one Sigmoid covers all four chunks
                    # (g weights pre-doubled; tanh(g) = 2*sigmoid(2g)-1)
                    gates = gatesp.tile([P, 4 * B], f32, tag="ps",
                                        name=f"gates_{l}_{j}_{t}")
                    for i in range(4):
                        nc.tensor.matmul(
                            out=gates[:, i * B:(i + 1) * B],
                            lhsT=bias_sb[l][0:1, i * H:(i + 1) * H],
                            rhs=ones_sb[0:1, 0:B], start=True, stop=False)
                        nc.tensor.matmul(
                            out=gates[:, i * B:(i + 1) * B],
                            lhsT=wih_sb[l][:, i * H:(i + 1) * H],
                            rhs=xin_l[:], start=False, stop=(t == 0))
                        if t > 0:
                            nc.tensor.matmul(
                                out=gates[:, i * B:(i + 1) * B],
                                lhsT=whh_sb[l][:, i * H:(i + 1) * H],
                                rhs=h[l][j][:], start=False, stop=True)
                    sg = ifop.tile([P, 4 * B], bf16, tag="ifo", name=f"sg_{l}_{j}_{t}")
                    nc.scalar.activation(sg[:], gates[:], AF.Sigmoid)
                    gt = gtp.tile([P, B], bf16, tag="gt", name=f"gt_{l}_{j}_{t}")
                    nc.vector.tensor_scalar(
                        out=gt[:], in0=sg[:, 3 * B:4 * B], scalar1=2.0, scalar2=-1.0,
                        op0=ALU.mult, op1=ALU.add)
                    if t == 0:
                        nc.vector.tensor_mul(c[l][j][:], sg[:, 0:B], gt[:])
                    else:
                        t1 = t1p.tile([P, B], f32, tag="t1", name=f"t1_{l}_{j}_{t}")
                        nc.vector.tensor_mul(t1[:], sg[:, B:2 * B], c[l][j][:])
                        t2 = t2p.tile([P, B], bf16, tag="t2", name=f"t2_{l}_{j}_{t}")
                        nc.vector.tensor_mul(t2[:], sg[:, 0:B], gt[:])
                        nc.gpsimd.tensor_tensor(c[l][j][:], t1[:], t2[:], op=ALU.add)
                    tcl = tclp.tile([P, B], bf16, tag="tc", name=f"tc_{l}_{j}_{t}")
                    nc.scalar.activation(tcl[:], c[l][j][:], AF.Tanh)
                    nc.vector.tensor_mul(h[l][j][:], sg[:, 2 * B:3 * B], tcl[:])

                def lstm_batch(tj_list):
                    # emit a batch of cell-steps: x loads, then all L0, then all L1+out
                    xs = {}
                    for t, j in tj_list:
                        B = LSTM_TILES[j][1]
                        x = xinp.tile([P, B], bf16, tag="x", name=f"x_{t}_{j}")
                        nc.sync.dma_start(
                            x[:], x0p[j][:].rearrange("h (t r) -> h t r", t=T)
                            [:, t:t + 1, 0:B].rearrange("h t r -> h (t r)"))
                        xs[(t, j)] = x
                    for t, j in tj_list:
                        cell(0, j, t, xs[(t, j)], LSTM_TILES[j][1])
                    for t, j in tj_list:
                        r0, B = LSTM_TILES[j]
                        cell(1, j, t, h[0][j], B)
                        y = youtp.tile([P, B], bf16, tag="y", name=f"y_{t}_{j}")
                        nc.vector.tensor_copy(y[:], h[1][j][:])
                        nc.sync.dma_start(
                            outr[:, t:t + 1, r0:r0 + B].rearrange("h t r -> h (t r)"), y[:])

                for d in range(N_TILES):
                    conv_tile(d)
                for t in range(T):
                    lstm_batch([(t, j) for j in range(NJ)])

    nc.compile()
    return nc


# ----------------------------------------------------------------------------
# Entry points
# ----------------------------------------------------------------------------

def _assemble(results):
    # per-core out: (128, T*R_CORE) viewed [h, t*R_CORE + r] -> (r, t, h)
    full = np.empty((N_NODE + N_POD + N_SVC, T, H), dtype=np.float32)
    parts_node, parts_pod, parts_svc = [], [], []
    for cidx, res in enumerate(results):
        o = res["out"].astype(np.float32).reshape(H, T, R_CORE).transpose(2, 1, 0)  # (r, t, h)
        n_node = min(NODE_PC, max(0, N_NODE - cidx * NODE_PC))
        n_svc = min(SVC_PC, max(0, N_SVC - cidx * SVC_PC))
        parts_pod.append(o[0:POD_PC])
        svc0 = POD_TILES * P
        parts_svc.append(o[svc0:svc0 + n_svc])
        node0 = (POD_TILES + SVC_TILES) * P
        parts_node.append(o[node0:node0 + n_node])
    full[0:N_NODE] = np.concatenate(parts_node, axis=0)
    full[N_NODE:N_NODE + N_POD] = np.concatenate(parts_pod, axis=0)
    full[N_NODE + N_POD:] = np.concatenate(parts_svc, axis=0)
    return full


def run(inputs, trace=False):
    from concourse.bass_utils import run_bass_kernel_spmd
    meta, in_maps = _prep(inputs)
    if meta not in _COMPILED:
        _COMPILED[meta] = _build(meta)
    nc = _COMPILED[meta]
    try:
        res = run_bass_kernel_spmd(nc, in_maps, core_ids=list(range(NCORES)), trace=trace)
    except Exception:
        # transient device errors (e.g. NRT_EXEC_UNIT_UNRECOVERABLE) recover
        # on re-execution; retry once before giving up
        res = run_bass_kernel_spmd(nc, in_maps, core_ids=list(range(NCORES)), trace=trace)
    return _assemble(res.results), res


def kernel(**inputs):
    out, _ = run(inputs, trace=False)
    return out


# revision 35
# speedup vs baseline: 1.0042x; 1.0042x over previous
"""Trainium2 Bass kernel for nn_AggrHGraphConvWindow (3x GraphConv -> LeakyReLU -> 2-layer LSTM).

Contract: kernel(**inputs) takes FULL unsharded numpy inputs, returns FULL output
(33500, 16, 128) float32.  Internally shards destination rows across 8 NeuronCores
(graph/data parallel per the sharding hint: edges partitioned by destination with
halo exchange of source features), runs one SPMD Bass program, and gathers.
"""

import os
import numpy as np
import ml_dtypes

BF16 = np.float16  # fp16: same cost as bf16 on PE/DVE, 8x finer mantissa

# Problem constants (hardcoded per spec)
N_NODE, N_POD, N_SVC = 500, 30000, 3000
T, F, H = 16, 64, 128
NCORES = 8
P = 128

NODE_PC = 64     # nodes per core (64*8=512 >= 500)
POD_PC = 3750    # pods per core (exact)
SVC_PC = 376     # svcs per core (376*8=3008 >= 3000)

NODE_TILES = 1   # 64 real rows inside one 128-row tile
POD_TILES = (POD_PC + P - 1) // P   # 30
SVC_TILES = (SVC_PC + P - 1) // P   # 3
N_TILES = NODE_TILES + POD_TILES + SVC_TILES  # 34
R_CORE = N_TILES * P  # 4352 rows per core (padded)

# LSTM batch tiles over the 4352 local rows
LSTM_TILES = [(j * 512, 512) for j in range(R_CORE // 512)]
if R_CORE % 512:
    LSTM_TILES.append((512 * (R_CORE // 512), R_CORE % 512))

_COMPILED = {}


# ----------------------------------------------------------------------------
# Host-side preprocessing: edge routing, degree norms, halo tables, weight prep
# ----------------------------------------------------------------------------

def _degrees(src, dst, n_src, n_dst):
    dout = np.bincount(src, minlength=n_src).astype(np.float64)
    din = np.bincount(dst, minlength=n_dst).astype(np.float64)
    return (1.0 / np.sqrt(np.maximum(dout, 1.0)), 1.0 / np.sqrt(np.maximum(din, 1.0)))


def _prep(inputs):
    nf = np.asarray(inputs["node_feat"]).reshape(N_NODE, T * F)
    pf = np.asarray(inputs["pod_feat"]).reshape(N_POD, T * F)
    sf = np.asarray(inputs["svc_feat"]).reshape(N_SVC, T * F)

    in_src = np.asarray(inputs["inst_node_src"]).astype(np.int64)
    in_dst = np.asarray(inputs["inst_node_dst"]).astype(np.int64)
    ni_src = np.asarray(inputs["node_inst_src"]).astype(np.int64)
    ni_dst = np.asarray(inputs["node_inst_dst"]).astype(np.int64)
    sc_src = np.asarray(inputs["svc_call_src"]).astype(np.int64)
    sc_dst = np.asarray(inputs["svc_call_dst"]).astype(np.int64)

    # normalization: x/sqrt(deg_out) -> segsum -> /sqrt(deg_in), folded per-edge
    ro_in, ri_in = _degrees(in_src, in_dst, N_POD, N_NODE)
    ro_ni, ri_ni = _degrees(ni_src, ni_dst, N_NODE, N_POD)
    ro_sc, ri_sc = _degrees(sc_src, sc_dst, N_SVC, N_SVC)

    # Route edges: per (core, tile) buckets.
    # tile order within core: pods tiles 0..29, svc 30..32, node 33 (node last)
    def route(src, dst, w, kind):
        if kind == 0:    # dst = node -> last tile (heaviest; keeps LSTM ramp fast)
            core = dst // NODE_PC
            q = dst - core * NODE_PC
            tile = np.full_like(dst, POD_TILES + SVC_TILES)
            row = q
        elif kind == 1:  # dst = pod -> tiles [0, POD_TILES)
            core = dst // POD_PC
            q = dst - core * POD_PC
            tile = q // P
            row = q % P
        else:            # dst = svc -> tiles [POD_TILES, POD_TILES+SVC_TILES)
            core = dst // SVC_PC
            q = dst - core * SVC_PC
            tile = POD_TILES + q // P
            row = q % P
        return core, tile, row, src, w

    ew_in = (ro_in[in_src] * ri_in[in_dst]).astype(np.float32)
    ew_ni = (ro_ni[ni_src] * ri_ni[ni_dst]).astype(np.float32)
    ew_sc = (ro_sc[sc_src] * ri_sc[sc_dst]).astype(np.float32)

    routed = {
        0: route(in_src, in_dst, ew_in, 0),   # node phase: src = pods
        1: route(ni_src, ni_dst, ew_ni, 1),   # pod phase:  src = nodes
        2: route(sc_src, sc_dst, ew_sc, 2),   # svc phase:  src = svcs
    }

    # per (core, tile) edge lists
    buckets = [[([], [], []) for _ in range(N_TILES)] for _ in range(NCORES)]
    for kind in (0, 1, 2):
        core, tile, row, src, w = routed[kind]
        order = np.lexsort((row, tile, core))
        core, tile, row, src, w = core[order], tile[order], row[order], src[order], w[order]
        # group
        key = core * N_TILES + tile
        uniq, starts = np.unique(key, return_index=True)
        starts = list(starts) + [len(key)]
        for ui, k in enumerate(uniq):
            c, t = int(k) // N_TILES, int(k) % N_TILES
            s, e = starts[ui], starts[ui + 1]
            buckets[c][t] = (src[s:e], row[s:e], w[s:e])

    # static chunk counts per tile (max over cores), >= 1
    K = []
    for t in range(N_TILES):
        mx = 1
        for c in range(NCORES):
            mx = max(mx, (len(buckets[c][t][0]) + P - 1) // P)
        K.append(mx)
    base = np.concatenate([[0], np.cumsum(K)]).astype(np.int64)
    C_total = int(base[-1])

    # halo source tables (bf16) + remapped edge arrays
    PN = K[-1] * P  # node-phase table rows (pods referenced by this core's edges)
    PS = sum(K[POD_TILES:POD_TILES + SVC_TILES]) * P  # svc-phase table rows
    nodetab = np.zeros((512, T * F), dtype=BF16)
    nodetab[:N_NODE] = nf.astype(BF16)

    in_maps = []
    for c in range(NCORES):
        esrc = np.zeros((C_total, P), dtype=np.int32)
        edst = np.zeros((C_total, P), dtype=np.float32)
        ew = np.zeros((C_total, P), dtype=np.float32)

        # node phase: compact pod rows
        NODE_TILE = POD_TILES + SVC_TILES
        psrc, prow, pw = buckets[c][NODE_TILE]
        uniq, inv = np.unique(psrc, return_inverse=True) if len(psrc) else (np.zeros(0, np.int64), np.zeros(0, np.int64))
        podtab = np.zeros((PN, T * F), dtype=BF16)
        podtab[:len(uniq)] = pf[uniq].astype(BF16)

        svctab = np.zeros((PS, T * F), dtype=BF16)
        svc_fill = 0

        for t in range(N_TILES):
            src, row, w = buckets[c][t]
            n = len(src)
            if t == NODE_TILE:
                sidx = inv
            elif t < POD_TILES:
                sidx = src  # direct index into nodetab
            else:
                u2, i2 = np.unique(src, return_inverse=True) if n else (np.zeros(0, np.int64), np.zeros(0, np.int64))
                svctab[svc_fill:svc_fill + len(u2)] = sf[u2].astype(BF16)
                sidx = i2 + svc_fill
                svc_fill += len(u2)
            b0 = int(base[t]) * P
            esrc.reshape(-1)[b0:b0 + n] = sidx
            edst.reshape(-1)[b0:b0 + n] = row
            ew.reshape(-1)[b0:b0 + n] = w

        m = {
            "podtab": podtab, "nodetab": nodetab, "svctab": svctab,
            "esrc": np.ascontiguousarray(esrc.T),
            "esrcf": np.ascontiguousarray(esrc.T.astype(np.float32)),
            "edst": np.ascontiguousarray(edst.T),
            "ew": np.ascontiguousarray(ew.T),
        }
        in_maps.append(m)

    # ---- weights (identical on all cores) ----
    def conv_w(Wname):
        W = np.asarray(inputs[Wname])  # (T, F, H)
        wt = W.transpose(1, 0, 2).reshape(F, T * H)  # (64, 2048) F-major
        return np.vstack([wt, wt]).astype(BF16)       # (128, 2048) vertical dup

    def conv_b(bname):
        return np.asarray(inputs[bname]).reshape(1, T * H).astype(BF16)

    def lstm_w(Wname):
        # rows [i,f,g,o] -> [i,f,o,g]; g block doubled so tanh(g) = 2*sigmoid(2g)-1
        # lets one Sigmoid cover all four gate chunks.
        W = np.asarray(inputs[Wname])  # (512, in_dim)
        Wp = np.concatenate([W[0:128], W[128:256], W[384:512], 2.0 * W[256:384]], axis=0)
        return np.ascontiguousarray(Wp.T).astype(BF16)  # (in_dim, 512), [i,f,o,2g]

    def lstm_b(b1, b2):
        b = np.asarray(inputs[b1]) + np.asarray(inputs[b2])
        bp = np.concatenate([b[0:128], b[128:256], b[384:512], 2.0 * b[256:384]])
        return bp.reshape(1, 512).astype(BF16)

    shared = {
        "wt_node": conv_w("W_in"), "wt_pod": conv_w("W_ni"), "wt_svc": conv_w("W_svc"),
        "bt_node": conv_b("b_in"), "bt_pod": conv_b("b_ni"), "bt_svc": conv_b("b_svc"),
        "wih0": lstm_w("Wih0"), "whh0": lstm_w("Whh0"),
        "wih1": lstm_w("Wih1"), "whh1": lstm_w("Whh1"),
        "bias0": lstm_b("bih0", "bhh0"), "bias1": lstm_b("bih1", "bhh1"),
        "iota": np.broadcast_to(np.arange(P, dtype=np.float32), (P, P)).copy(),
        "iota512": np.broadcast_to(np.arange(512, dtype=np.float32), (P, 512)).copy(),
    }
    for m in in_maps:
        m.update(shared)

    meta = (C_total, tuple(K), PN, PS)
    return meta, in_maps


# ----------------------------------------------------------------------------
# Device program
# ----------------------------------------------------------------------------

def _build(meta):
    import concourse.bass as bass
    import concourse.tile as tile
    import concourse.mybir as mybir

    C_total, K, PN, PS = meta
    f32 = mybir.dt.float32
    bf16 = mybir.dt.float16
    i32 = mybir.dt.int32
    AF = mybir.ActivationFunctionType
    ALU = mybir.AluOpType

    import concourse.bacc as bacc
    nc = bacc.Bacc("TRN2", target_bir_lowering=False, debug=False, enable_asserts=False)

    podtab = nc.dram_tensor("podtab", [PN, T * F], bf16, kind="ExternalInput")
    nodetab = nc.dram_tensor("nodetab", [512, T * F], bf16, kind="ExternalInput")
    svctab = nc.dram_tensor("svctab", [PS, T * F], bf16, kind="ExternalInput")
    esrc_d = nc.dram_tensor("esrc", [P, C_total], i32, kind="ExternalInput")
    esrcf_d = nc.dram_tensor("esrcf", [P, C_total], f32, kind="ExternalInput")
    edst_d = nc.dram_tensor("edst", [P, C_total], f32, kind="ExternalInput")
    ew_d = nc.dram_tensor("ew", [P, C_total], f32, kind="ExternalInput")
    wt_d = {k: nc.dram_tensor(f"wt_{k}", [P, T * H], bf16, kind="ExternalInput")
            for k in ("node", "pod", "svc")}
    bt_d = {k: nc.dram_tensor(f"bt_{k}", [1, T * H], bf16, kind="ExternalInput")
            for k in ("node", "pod", "svc")}
    wih_d = [nc.dram_tensor(f"wih{l}", [H, 512], bf16, kind="ExternalInput") for l in range(2)]
    whh_d = [nc.dram_tensor(f"whh{l}", [H, 512], bf16, kind="ExternalInput") for l in range(2)]
    bias_d = [nc.dram_tensor(f"bias{l}", [1, 512], bf16, kind="ExternalInput") for l in range(2)]
    iota_d = nc.dram_tensor("iota", [P, P], f32, kind="ExternalInput")
    iota512_d = nc.dram_tensor("iota512", [P, 512], f32, kind="ExternalInput")
    out_d = nc.dram_tensor("out", [P, T * R_CORE], bf16, kind="ExternalOutput")

    tile_kind = (["pod"] * POD_TILES + ["svc"] * SVC_TILES + ["node"] * NODE_TILES)
    base = np.concatenate([[0], np.cumsum(K)]).astype(int)

    NJ = len(LSTM_TILES)
    with tile.TileContext(nc) as tc:
        with tc.tile_pool(name="dram", bufs=NJ, space="DRAM") as dramp, \
             tc.tile_pool(name="const", bufs=1) as constp:
            # x0 spill split per LSTM batch tile so the LSTM can start on tile j
            # as soon as its 4 conv row-tiles are written (pipeline the phases)
            x0p = [dramp.tile([P, T * B], bf16, tag="x0p", name=f"x0p_{j}")
                   for j, (r0, B) in enumerate(LSTM_TILES)]

            # load constants
            esrc_sb = constp.tile([P, C_total], i32)
            esrcf_sb = constp.tile([P, C_total], f32)
            edst_sb = constp.tile([P, C_total], f32)
            ew_sb = constp.tile([P, C_total], f32)
            iota_sb = constp.tile([P, P], f32)
            iota512_sb = constp.tile([P, 512], f32)
            nc.sync.dma_start(esrc_sb[:], esrc_d.ap())
            nc.sync.dma_start(esrcf_sb[:], esrcf_d.ap())
            nc.sync.dma_start(edst_sb[:], edst_d.ap())
            nc.sync.dma_start(ew_sb[:], ew_d.ap())
            nc.sync.dma_start(iota_sb[:], iota_d.ap())
            nc.sync.dma_start(iota512_sb[:], iota512_d.ap())
            # node feature table resident in SBUF for the pod-phase one-hot
            # matmul conv (replaces 15MB of per-edge row gathers)
            ntab_sb = []
            for sc in range(4):
                ntab_sb.append(constp.tile([P, T * F], bf16, name=f"ntab_{sc}"))
                nc.sync.dma_start(ntab_sb[sc][:], nodetab.ap()[sc * P:(sc + 1) * P, :])
            wt_sb, bt_sb = {}, {}
            for k in ("node", "pod", "svc"):
                wt_sb[k] = constp.tile([P, T * H], bf16, name=f"wt_{k}_sb")
                bt_sb[k] = constp.tile([1, T * H], bf16, name=f"bt_{k}_sb")
                nc.sync.dma_start(wt_sb[k][:], wt_d[k].ap())
                nc.sync.dma_start(bt_sb[k][:], bt_d[k].ap())
            wih_sb, whh_sb, bias_sb = [], [], []
            for l in range(2):
                wih_sb.append(constp.tile([H, 512], bf16, name=f"wih{l}_sb"))
                whh_sb.append(constp.tile([H, 512], bf16, name=f"whh{l}_sb"))
                bias_sb.append(constp.tile([1, 512], bf16, name=f"bias{l}_sb"))
                nc.sync.dma_start(wih_sb[l][:], wih_d[l].ap())
                nc.sync.dma_start(whh_sb[l][:], whh_d[l].ap())
                nc.sync.dma_start(bias_sb[l][:], bias_d[l].ap())
            ones_sb = constp.tile([1, 512], bf16)
            nc.gpsimd.memset(ones_sb[:], 1.0)

            srctabs = {"node": podtab, "pod": nodetab, "svc": svctab}

            # Conv + LSTM share one scope (and one PSUM pool) so the two
            # phases pipeline: LSTM batch-tile j starts once its 4 conv
            # row-tiles have spilled.
            with tc.tile_pool(name="gat", bufs=6) as gatp, \
                 tc.tile_pool(name="ssb", bufs=6) as ssbp, \
                 tc.tile_pool(name="psum", bufs=2, space="PSUM") as psump, \
                 tc.tile_pool(name="aggsb", bufs=3) as aggsbp, \
                 tc.tile_pool(name="x0sb", bufs=3) as x0sbp, \
                 tc.tile_pool(name="x0res", bufs=2) as x0resp, \
                 tc.tile_pool(name="st_h0", bufs=NJ) as ph0, \
                 tc.tile_pool(name="st_c0", bufs=NJ) as pc0, \
                 tc.tile_pool(name="st_h1", bufs=NJ) as ph1, \
                 tc.tile_pool(name="st_c1", bufs=NJ) as pc1, \
                 tc.tile_pool(name="ifo", bufs=4) as ifop, \
                 tc.tile_pool(name="gt", bufs=4) as gtp, \
                 tc.tile_pool(name="tcl", bufs=4) as tclp, \
                 tc.tile_pool(name="tmp1", bufs=2) as t1p, \
                 tc.tile_pool(name="tmp2", bufs=2) as t2p, \
                 tc.tile_pool(name="xin", bufs=6) as xinp, \
                 tc.tile_pool(name="yout", bufs=3) as youtp:

                def conv_group(d, k0, k1, gs, ss, agg):
                    # PE-accumulate chunk group [k0,k1] into one PSUM partial
                    # (m-outer so per-bank psum accumulation groups stay
                    # sequential), then DVE-combine into agg.
                    pp = psump.tile([P, T * H], f32, tag="ps", name=f"pp_{d}_{k0}")
                    for m in range(8):
                        for kk in range(k0, k1 + 1):
                            nc.tensor.matmul(
                                out=pp[:, m * P:(m + 1) * P],
                                lhsT=gs[kk][:, m * P:(m + 1) * P],
                                rhs=ss[kk][:], start=(kk == k0), stop=(kk == k1))
                    if k0 == 0:
                        nc.vector.tensor_copy(agg[:], pp[:, 0:T * F])
                    else:
                        nc.vector.tensor_tensor(out=agg[:], in0=agg[:],
                                                in1=pp[:, 0:T * F], op=ALU.add)

                N_RES = 2  # LSTM batch tiles whose x0 stays SBUF-resident
                x0res = [x0resp.tile([P, T * 512], bf16, tag="x0r", name=f"x0res_{j}")
                         for j in range(N_RES)]

                def conv_tail(d, kind, agg):
                    # linear per timestep + bias (rank-1 matmul) -> PSUM,
                    # then LeakyReLU(0.01) -> fp16 -> spill (or SBUF-resident)
                    hx = psump.tile([P, T * H], f32, tag="ps", name=f"hx_{d}")
                    wt, bt = wt_sb[kind], bt_sb[kind]
                    for t in range(T):
                        pb = 64 * (t % 2)
                        nc.tensor.matmul(
                            out=hx[:, t * H:(t + 1) * H], lhsT=bt[0:1, t * H:(t + 1) * H],
                            rhs=ones_sb[0:1, 0:P], start=True, stop=False)
                        nc.tensor.matmul(
                            out=hx[:, t * H:(t + 1) * H],
                            lhsT=wt[pb:pb + F, t * H:(t + 1) * H],
                            rhs=agg[pb:pb + F, (t // 2) * P:(t // 2 + 1) * P],
                            start=False, stop=True)
                    j = min(d // 4, NJ - 1)
                    rl = P * (d - 4 * j)
                    if j < N_RES:
                        # Prelu writes straight into the resident tile's
                        # [h, t*512 + rl + r] slices; no DRAM round trip
                        dst = x0res[j][:].rearrange("h (t r) -> h t r", t=T)[:, :, rl:rl + P]
                        nc.scalar.activation(
                            dst, hx[:].rearrange("h (t r) -> h t r", t=T), AF.Prelu, alpha=0.01)
                        return
                    x0t = x0sbp.tile([P, T * H], bf16, tag="x0", name=f"x0t_{d}")
                    nc.scalar.activation(x0t[:], hx[:], AF.Prelu, alpha=0.01)
                    nc.sync.dma_start(
                        x0p[j][:].rearrange("h (t r) -> h t r", t=T)[:, :, rl:rl + P],
                        x0t[:].rearrange("h (t r) -> h t r", t=T))

                def conv_tile_pod(d):
                    # Pod conv via one-hot matmuls against the SBUF-resident
                    # node table: C[s,r] = sum_e w_e [src=s][dst=r], then
                    # agg.T = nodetab.T @ C. No DRAM gather at all.
                    Kd = K[d]
                    agg = aggsbp.tile([P, T * F], bf16, tag="agg", name=f"agg_{d}")
                    ohs, s0s = [], []
                    for ki in range(Kd):
                        col = int(base[d]) + ki
                        oh = gatp.tile([P, 512], bf16, tag="g", name=f"oh_{d}_{ki}")
                        nc.vector.tensor_scalar(
                            out=oh[:], in0=iota512_sb[:],
                            scalar1=esrcf_sb[:, col:col + 1], scalar2=ew_sb[:, col:col + 1],
                            op0=ALU.is_equal, op1=ALU.mult)
                        s0 = ssbp.tile([P, P], bf16, tag="s", name=f"s0_{d}_{ki}")
                        nc.vector.tensor_scalar(
                            out=s0[:], in0=iota_sb[:],
                            scalar1=edst_sb[:, col:col + 1], scalar2=None,
                            op0=ALU.is_equal)
                        ohs.append(oh)
                        s0s.append(s0)
                    cps = psump.tile([P, T * H], f32, tag="ps", name=f"cps_{d}")
                    for sc in range(4):
                        for ki in range(Kd):
                            nc.tensor.matmul(
                                out=cps[:, sc * P:(sc + 1) * P],
                                lhsT=ohs[ki][:, sc * P:(sc + 1) * P], rhs=s0s[ki][:],
                                start=(ki == 0), stop=(ki == Kd - 1))
                    csb = ssbp.tile([P, 512], bf16, tag="csb", name=f"csb_{d}")
                    nc.vector.tensor_copy(csb[:], cps[:, 0:512])
                    # reuse the same PSUM tile for agg (start=True overwrites);
                    # keeps pod tiles at 2 psum allocations for 2-slot pipelining
                    for m in range(8):
                        for sc in range(4):
                            nc.tensor.matmul(
                                out=cps[:, m * P:(m + 1) * P],
                                lhsT=ntab_sb[sc][:, m * P:(m + 1) * P],
                                rhs=csb[:, sc * P:(sc + 1) * P],
                                start=(sc == 0), stop=(sc == 3))
                    nc.vector.tensor_copy(agg[:], cps[:, 0:T * F])
                    conv_tail(d, "pod", agg)

                def conv_tile(d):
                    kind = tile_kind[d]
                    Kd = K[d]
                    tab = srctabs[kind]
                    agg = aggsbp.tile([P, T * F], bf16, tag="agg", name=f"agg_{d}")
                    gs, ss = [], []
                    for ki in range(Kd):
                        col = int(base[d]) + ki
                        g = gatp.tile([P, T * F], bf16, tag="g", name=f"g_{d}_{ki}")
                        nc.gpsimd.indirect_dma_start(
                            out=g[:], out_offset=None, in_=tab.ap(),
                            in_offset=bass.IndirectOffsetOnAxis(ap=esrc_sb[:, col:col + 1], axis=0))
                        s = ssbp.tile([P, P], bf16, tag="s", name=f"s_{d}_{ki}")
                        nc.vector.tensor_scalar(
                            out=s[:], in0=iota_sb[:],
                            scalar1=edst_sb[:, col:col + 1], scalar2=ew_sb[:, col:col + 1],
                            op0=ALU.is_equal, op1=ALU.mult)
                        gs.append(g)
                        ss.append(s)
                        if ki % 4 == 3 or ki == Kd - 1:
                            conv_group(d, (ki // 4) * 4, ki, gs, ss, agg)
                    conv_tail(d, kind, agg)

                # ---------------- LSTM phase ----------------
                gatesp = psump

                h = [[None] * NJ, [None] * NJ]
                c = [[None] * NJ, [None] * NJ]
                for j, (r0, B) in enumerate(LSTM_TILES):
                    h[0][j] = ph0.tile([P, B], bf16, tag="h0", name=f"h0_{j}")
                    c[0][j] = pc0.tile([P, B], f32, tag="c0", name=f"c0_{j}")
                    h[1][j] = ph1.tile([P, B], bf16, tag="h1", name=f"h1_{j}")
                    c[1][j] = pc1.tile([P, B], f32, tag="c1", name=f"c1_{j}")

                outr = out_d.ap().rearrange("h (t r) -> h t r", t=T)

                def cell(l, j, t, xin_l, B):
                    # gates [i,f,o,2g]; one Sigmoid covers all four chunks
                    # (g weights pre-doubled; tanh(g) = 2*sigmoid(2g)-1)
                    gates = gatesp.tile([P, 4 * B], f32, tag="ps",
                                        name=f"gates_{l}_{j}_{t}")
                    for i in range(4):
                        nc.tensor.matmul(
                            out=gates[:, i * B:(i + 1) * B],
                            lhsT=bias_sb[l][0:1, i * H:(i + 1) * H],
                            rhs=ones_sb[0:1, 0:B], start=True, stop=False)
                        nc.tensor.matmul(
                            out=gates[:, i * B:(i + 1) * B],
                            lhsT=wih_sb[l][:, i * H:(i + 1) * H],
                            rhs=xin_l[:], start=False, stop=(t == 0))
                        if t > 0:
                            nc.tensor.matmul(
                                out=gates[:, i * B:(i + 1) * B],
                                lhsT=whh_sb[l][:, i * H:(i + 1) * H],
                                rhs=h[l][j][:], start=False, stop=True)
                    sg = ifop.tile([P, 4 * B], bf16, tag="ifo", name=f"sg_{l}_{j}_{t}")
                    nc.scalar.activation(sg[:], gates[:], AF.Sigmoid)
                    gt = gtp.tile([P, B], bf16, tag="gt", name=f"gt_{l}_{j}_{t}")
                    nc.vector.tensor_scalar(
                        out=gt[:], in0=sg[:, 3 * B:4 * B], scalar1=2.0, scalar2=-1.0,
                        op0=ALU.mult, op1=ALU.add)
                    if t == 0:
                        nc.vector.tensor_mul(c[l][j][:], sg[:, 0:B], gt[:])
                    else:
                        t1 = t1p.tile([P, B], f32, tag="t1", name=f"t1_{l}_{j}_{t}")
                        nc.vector.tensor_mul(t1[:], sg[:, B:2 * B], c[l][j][:])
                        t2 = t2p.tile([P, B], bf16, tag="t2", name=f"t2_{l}_{j}_{t}")
                        nc.vector.tensor_mul(t2[:], sg[:, 0:B], gt[:])
                        nc.gpsimd.tensor_tensor(c[l][j][:], t1[:], t2[:], op=ALU.add)
                    tcl = tclp.tile([P, B], bf16, tag="tc", name=f"tc_{l}_{j}_{t}")
                    nc.scalar.activation(tcl[:], c[l][j][:], AF.Tanh)
                    nc.vector.tensor_mul(h[l][j][:], sg[:, 2 * B:3 * B], tcl[:])

                def lstm_batch(tj_list):
                    # emit a batch of cell-steps: x loads, then all L0, then all L1+out
                    xs = {}
                    for t, j in tj_list:
                        if j < 2:
                            xs[(t, j)] = x0res[j][:, t * 512:(t + 1) * 512]
                            continue
                        B = LSTM_TILES[j][1]
                        x = xinp.tile([P, B], bf16, tag="x", name=f"x_{t}_{j}")
                        nc.sync.dma_start(
                            x[:], x0p[j][:].rearrange("h (t r) -> h t r", t=T)
                            [:, t:t + 1, 0:B].rearrange("h t r -> h (t r)"))
                        xs[(t, j)] = x
                    for t, j in tj_list:
                        cell(0, j, t, xs[(t, j)], LSTM_TILES[j][1])
                    for t, j in tj_list:
                        r0, B = LSTM_TILES[j]
                        cell(1, j, t, h[0][j], B)
                        y = youtp.tile([P, B], bf16, tag="y", name=f"y_{t}_{j}")
                        nc.vector.tensor_copy(y[:], h[1][j][:])
                        nc.sync.dma_start(
                            outr[:, t:t + 1, r0:r0 + B].rearrange("h t r -> h (t r)"), y[:])

                for d in range(N_TILES):
                    conv_tile(d)
                for t in range(T):
                    lstm_batch([(t, j) for j in range(NJ)])

    nc.compile()
    return nc


# ----------------------------------------------------------------------------
# Entry points
# ----------------------------------------------------------------------------

def _assemble(results):
    # per-core out: (128, T*R_CORE) viewed [h, t*R_CORE + r] -> (r, t, h)
    full = np.empty((N_NODE + N_POD + N_SVC, T, H), dtype=np.float32)
    parts_node, parts_pod, parts_svc = [], [], []
    for cidx, res in enumerate(results):
        o = res["out"].astype(np.float32).reshape(H, T, R_CORE).transpose(2, 1, 0)  # (r, t, h)
        n_node = min(NODE_PC, max(0, N_NODE - cidx * NODE_PC))
        n_svc = min(SVC_PC, max(0, N_SVC - cidx * SVC_PC))
        parts_pod.append(o[0:POD_PC])
        svc0 = POD_TILES * P
        parts_svc.append(o[svc0:svc0 + n_svc])
        node0 = (POD_TILES + SVC_TILES) * P
        parts_node.append(o[node0:node0 + n_node])
    full[0:N_NODE] = np.concatenate(parts_node, axis=0)
    full[N_NODE:N_NODE + N_POD] = np.concatenate(parts_pod, axis=0)
    full[N_NODE + N_POD:] = np.concatenate(parts_svc, axis=0)
    return full


def run(inputs, trace=False):
    from concourse.bass_utils import run_bass_kernel_spmd
    meta, in_maps = _prep(inputs)
    if meta not in _COMPILED:
        _COMPILED[meta] = _build(meta)
    nc = _COMPILED[meta]
    try:
        res = run_bass_kernel_spmd(nc, in_maps, core_ids=list(range(NCORES)), trace=trace)
    except Exception:
        # transient device errors (e.g. NRT_EXEC_UNIT_UNRECOVERABLE) recover
        # on re-execution; retry once before giving up
        res = run_bass_kernel_spmd(nc, in_maps, core_ids=list(range(NCORES)), trace=trace)
    return _assemble(res.results), res


def kernel(**inputs):
    out, _ = run(inputs, trace=False)
    return out


# revision 37
# speedup vs baseline: 1.0147x; 1.0104x over previous
"""Trainium2 Bass kernel for nn_AggrHGraphConvWindow (3x GraphConv -> LeakyReLU -> 2-layer LSTM).

Contract: kernel(**inputs) takes FULL unsharded numpy inputs, returns FULL output
(33500, 16, 128) float32.  Internally shards destination rows across 8 NeuronCores
(graph/data parallel per the sharding hint: edges partitioned by destination with
halo exchange of source features), runs one SPMD Bass program, and gathers.
"""

import os
import numpy as np
import ml_dtypes

BF16 = np.float16  # fp16: same cost as bf16 on PE/DVE, 8x finer mantissa

# Problem constants (hardcoded per spec)
N_NODE, N_POD, N_SVC = 500, 30000, 3000
T, F, H = 16, 64, 128
NCORES = 8
P = 128

NODE_PC = 64     # nodes per core (64*8=512 >= 500)
POD_PC = 3750    # pods per core (exact)
SVC_PC = 376     # svcs per core (376*8=3008 >= 3000)

NODE_TILES = 1   # 64 real rows inside one 128-row tile
POD_TILES = (POD_PC + P - 1) // P   # 30
SVC_TILES = (SVC_PC + P - 1) // P   # 3
N_TILES = NODE_TILES + POD_TILES + SVC_TILES  # 34
R_CORE = N_TILES * P  # 4352 rows per core (padded)

# LSTM batch tiles over the 4352 local rows
LSTM_TILES = [(j * 512, 512) for j in range(R_CORE // 512)]
if R_CORE % 512:
    LSTM_TILES.append((512 * (R_CORE // 512), R_CORE % 512))

_COMPILED = {}


# ----------------------------------------------------------------------------
# Host-side preprocessing: edge routing, degree norms, halo tables, weight prep
# ----------------------------------------------------------------------------

def _degrees(src, dst, n_src, n_dst):
    dout = np.bincount(src, minlength=n_src).astype(np.float64)
    din = np.bincount(dst, minlength=n_dst).astype(np.float64)
    return (1.0 / np.sqrt(np.maximum(dout, 1.0)), 1.0 / np.sqrt(np.maximum(din, 1.0)))


def _prep(inputs):
    nf = np.asarray(inputs["node_feat"]).reshape(N_NODE, T * F)
    pf = np.asarray(inputs["pod_feat"]).reshape(N_POD, T * F)
    sf = np.asarray(inputs["svc_feat"]).reshape(N_SVC, T * F)

    in_src = np.asarray(inputs["inst_node_src"]).astype(np.int64)
    in_dst = np.asarray(inputs["inst_node_dst"]).astype(np.int64)
    ni_src = np.asarray(inputs["node_inst_src"]).astype(np.int64)
    ni_dst = np.asarray(inputs["node_inst_dst"]).astype(np.int64)
    sc_src = np.asarray(inputs["svc_call_src"]).astype(np.int64)
    sc_dst = np.asarray(inputs["svc_call_dst"]).astype(np.int64)

    # normalization: x/sqrt(deg_out) -> segsum -> /sqrt(deg_in), folded per-edge
    ro_in, ri_in = _degrees(in_src, in_dst, N_POD, N_NODE)
    ro_ni, ri_ni = _degrees(ni_src, ni_dst, N_NODE, N_POD)
    ro_sc, ri_sc = _degrees(sc_src, sc_dst, N_SVC, N_SVC)

    # Route edges: per (core, tile) buckets.
    # tile order within core: pods tiles 0..29, svc 30..32, node 33 (node last)
    def route(src, dst, w, kind):
        if kind == 0:    # dst = node -> last tile (heaviest; keeps LSTM ramp fast)
            core = dst // NODE_PC
            q = dst - core * NODE_PC
            tile = np.full_like(dst, POD_TILES + SVC_TILES)
            row = q
        elif kind == 1:  # dst = pod -> tiles [0, POD_TILES)
            core = dst // POD_PC
            q = dst - core * POD_PC
            tile = q // P
            row = q % P
        else:            # dst = svc -> tiles [POD_TILES, POD_TILES+SVC_TILES)
            core = dst // SVC_PC
            q = dst - core * SVC_PC
            tile = POD_TILES + q // P
            row = q % P
        return core, tile, row, src, w

    ew_in = (ro_in[in_src] * ri_in[in_dst]).astype(np.float32)
    ew_ni = (ro_ni[ni_src] * ri_ni[ni_dst]).astype(np.float32)
    ew_sc = (ro_sc[sc_src] * ri_sc[sc_dst]).astype(np.float32)

    routed = {
        0: route(in_src, in_dst, ew_in, 0),   # node phase: src = pods
        1: route(ni_src, ni_dst, ew_ni, 1),   # pod phase:  src = nodes
        2: route(sc_src, sc_dst, ew_sc, 2),   # svc phase:  src = svcs
    }

    # per (core, tile) edge lists
    buckets = [[([], [], []) for _ in range(N_TILES)] for _ in range(NCORES)]
    for kind in (0, 1, 2):
        core, tile, row, src, w = routed[kind]
        order = np.lexsort((row, tile, core))
        core, tile, row, src, w = core[order], tile[order], row[order], src[order], w[order]
        # group
        key = core * N_TILES + tile
        uniq, starts = np.unique(key, return_index=True)
        starts = list(starts) + [len(key)]
        for ui, k in enumerate(uniq):
            c, t = int(k) // N_TILES, int(k) % N_TILES
            s, e = starts[ui], starts[ui + 1]
            buckets[c][t] = (src[s:e], row[s:e], w[s:e])

    # static chunk counts per tile (max over cores), >= 1
    K = []
    for t in range(N_TILES):
        mx = 1
        for c in range(NCORES):
            mx = max(mx, (len(buckets[c][t][0]) + P - 1) // P)
        K.append(mx)
    base = np.concatenate([[0], np.cumsum(K)]).astype(np.int64)
    C_total = int(base[-1])

    # halo source tables (bf16) + remapped edge arrays
    PN = K[-1] * P  # node-phase table rows (pods referenced by this core's edges)
    PS = sum(K[POD_TILES:POD_TILES + SVC_TILES]) * P  # svc-phase table rows
    nodetab = np.zeros((512, T * F), dtype=BF16)
    nodetab[:N_NODE] = nf.astype(BF16)

    in_maps = []
    for c in range(NCORES):
        esrc = np.zeros((C_total, P), dtype=np.int32)
        edst = np.zeros((C_total, P), dtype=np.float32)
        ew = np.zeros((C_total, P), dtype=np.float32)

        # node phase: compact pod rows
        NODE_TILE = POD_TILES + SVC_TILES
        psrc, prow, pw = buckets[c][NODE_TILE]
        uniq, inv = np.unique(psrc, return_inverse=True) if len(psrc) else (np.zeros(0, np.int64), np.zeros(0, np.int64))
        podtab = np.zeros((PN, T * F), dtype=BF16)
        podtab[:len(uniq)] = pf[uniq].astype(BF16)

        svctab = np.zeros((PS, T * F), dtype=BF16)
        svc_fill = 0

        for t in range(N_TILES):
            src, row, w = buckets[c][t]
            n = len(src)
            if t == NODE_TILE:
                sidx = inv
            elif t < POD_TILES:
                sidx = src  # direct index into nodetab
            else:
                u2, i2 = np.unique(src, return_inverse=True) if n else (np.zeros(0, np.int64), np.zeros(0, np.int64))
                svctab[svc_fill:svc_fill + len(u2)] = sf[u2].astype(BF16)
                sidx = i2 + svc_fill
                svc_fill += len(u2)
            b0 = int(base[t]) * P
            esrc.reshape(-1)[b0:b0 + n] = sidx
            edst.reshape(-1)[b0:b0 + n] = row
            ew.reshape(-1)[b0:b0 + n] = w

        m = {
            "podtab": podtab, "nodetab": nodetab, "svctab": svctab,
            "esrc": np.ascontiguousarray(esrc.T),
            "esrcf": np.ascontiguousarray(esrc.T.astype(np.float32)),
            "edst": np.ascontiguousarray(edst.T),
            "ew": np.ascontiguousarray(ew.T),
        }
        in_maps.append(m)

    # ---- weights (identical on all cores) ----
    def conv_w(Wname):
        W = np.asarray(inputs[Wname])  # (T, F, H)
        wt = W.transpose(1, 0, 2).reshape(F, T * H)  # (64, 2048) F-major
        return np.vstack([wt, wt]).astype(BF16)       # (128, 2048) vertical dup

    def conv_b(bname):
        return np.asarray(inputs[bname]).reshape(1, T * H).astype(BF16)

    def lstm_w(Wname):
        # rows [i,f,g,o] -> [i,f,o,g]; g block doubled so tanh(g) = 2*sigmoid(2g)-1
        # lets one Sigmoid cover all four gate chunks.
        W = np.asarray(inputs[Wname])  # (512, in_dim)
        Wp = np.concatenate([W[0:128], W[128:256], W[384:512], 2.0 * W[256:384]], axis=0)
        return np.ascontiguousarray(Wp.T).astype(BF16)  # (in_dim, 512), [i,f,o,2g]

    def lstm_b(b1, b2):
        b = np.asarray(inputs[b1]) + np.asarray(inputs[b2])
        bp = np.concatenate([b[0:128], b[128:256], b[384:512], 2.0 * b[256:384]])
        return bp.reshape(1, 512).astype(BF16)

    shared = {
        "wt_node": conv_w("W_in"), "wt_pod": conv_w("W_ni"), "wt_svc": conv_w("W_svc"),
        "bt_node": conv_b("b_in"), "bt_pod": conv_b("b_ni"), "bt_svc": conv_b("b_svc"),
        "wih0": lstm_w("Wih0"), "whh0": lstm_w("Whh0"),
        "wih1": lstm_w("Wih1"), "whh1": lstm_w("Whh1"),
        "bias0": lstm_b("bih0", "bhh0"), "bias1": lstm_b("bih1", "bhh1"),
        "iota": np.broadcast_to(np.arange(P, dtype=np.float32), (P, P)).copy(),
        "iota512": np.broadcast_to(np.arange(512, dtype=np.float32), (P, 512)).copy(),
    }
    for m in in_maps:
        m.update(shared)

    meta = (C_total, tuple(K), PN, PS)
    return meta, in_maps


# ----------------------------------------------------------------------------
# Device program
# ----------------------------------------------------------------------------

def _build(meta):
    import concourse.bass as bass
    import concourse.tile as tile
    import concourse.mybir as mybir

    C_total, K, PN, PS = meta
    f32 = mybir.dt.float32
    bf16 = mybir.dt.float16
    i32 = mybir.dt.int32
    AF = mybir.ActivationFunctionType
    ALU = mybir.AluOpType

    import concourse.bacc as bacc
    nc = bacc.Bacc("TRN2", target_bir_lowering=False, debug=False, enable_asserts=False)

    podtab = nc.dram_tensor("podtab", [PN, T * F], bf16, kind="ExternalInput")
    nodetab = nc.dram_tensor("nodetab", [512, T * F], bf16, kind="ExternalInput")
    svctab = nc.dram_tensor("svctab", [PS, T * F], bf16, kind="ExternalInput")
    esrc_d = nc.dram_tensor("esrc", [P, C_total], i32, kind="ExternalInput")
    esrcf_d = nc.dram_tensor("esrcf", [P, C_total], f32, kind="ExternalInput")
    edst_d = nc.dram_tensor("edst", [P, C_total], f32, kind="ExternalInput")
    ew_d = nc.dram_tensor("ew", [P, C_total], f32, kind="ExternalInput")
    wt_d = {k: nc.dram_tensor(f"wt_{k}", [P, T * H], bf16, kind="ExternalInput")
            for k in ("node", "pod", "svc")}
    bt_d = {k: nc.dram_tensor(f"bt_{k}", [1, T * H], bf16, kind="ExternalInput")
            for k in ("node", "pod", "svc")}
    wih_d = [nc.dram_tensor(f"wih{l}", [H, 512], bf16, kind="ExternalInput") for l in range(2)]
    whh_d = [nc.dram_tensor(f"whh{l}", [H, 512], bf16, kind="ExternalInput") for l in range(2)]
    bias_d = [nc.dram_tensor(f"bias{l}", [1, 512], bf16, kind="ExternalInput") for l in range(2)]
    iota_d = nc.dram_tensor("iota", [P, P], f32, kind="ExternalInput")
    iota512_d = nc.dram_tensor("iota512", [P, 512], f32, kind="ExternalInput")
    out_d = nc.dram_tensor("out", [P, T * R_CORE], bf16, kind="ExternalOutput")

    tile_kind = (["pod"] * POD_TILES + ["svc"] * SVC_TILES + ["node"] * NODE_TILES)
    base = np.concatenate([[0], np.cumsum(K)]).astype(int)

    NJ = len(LSTM_TILES)
    with tile.TileContext(nc) as tc:
        with tc.tile_pool(name="dram", bufs=NJ, space="DRAM") as dramp, \
             tc.tile_pool(name="const", bufs=1) as constp:
            # x0 spill split per LSTM batch tile so the LSTM can start on tile j
            # as soon as its 4 conv row-tiles are written (pipeline the phases)
            x0p = [dramp.tile([P, T * B], bf16, tag="x0p", name=f"x0p_{j}")
                   for j, (r0, B) in enumerate(LSTM_TILES)]

            # load constants
            esrc_sb = constp.tile([P, C_total], i32)
            esrcf_sb = constp.tile([P, C_total], f32)
            edst_sb = constp.tile([P, C_total], f32)
            ew_sb = constp.tile([P, C_total], f32)
            iota_sb = constp.tile([P, P], f32)
            iota512_sb = constp.tile([P, 512], f32)
            nc.sync.dma_start(esrc_sb[:], esrc_d.ap())
            nc.sync.dma_start(esrcf_sb[:], esrcf_d.ap())
            nc.sync.dma_start(edst_sb[:], edst_d.ap())
            nc.sync.dma_start(ew_sb[:], ew_d.ap())
            nc.sync.dma_start(iota_sb[:], iota_d.ap())
            nc.sync.dma_start(iota512_sb[:], iota512_d.ap())
            # node feature table resident in SBUF for the pod-phase one-hot
            # matmul conv (replaces 15MB of per-edge row gathers)
            ntab_sb = []
            for sc in range(4):
                ntab_sb.append(constp.tile([P, T * F], bf16, name=f"ntab_{sc}"))
                nc.sync.dma_start(ntab_sb[sc][:], nodetab.ap()[sc * P:(sc + 1) * P, :])
            wt_sb, bt_sb = {}, {}
            for k in ("node", "pod", "svc"):
                wt_sb[k] = constp.tile([P, T * H], bf16, name=f"wt_{k}_sb")
                bt_sb[k] = constp.tile([1, T * H], bf16, name=f"bt_{k}_sb")
                nc.sync.dma_start(wt_sb[k][:], wt_d[k].ap())
                nc.sync.dma_start(bt_sb[k][:], bt_d[k].ap())
            wih_sb, whh_sb, bias_sb = [], [], []
            for l in range(2):
                wih_sb.append(constp.tile([H, 512], bf16, name=f"wih{l}_sb"))
                whh_sb.append(constp.tile([H, 512], bf16, name=f"whh{l}_sb"))
                bias_sb.append(constp.tile([1, 512], bf16, name=f"bias{l}_sb"))
                nc.sync.dma_start(wih_sb[l][:], wih_d[l].ap())
                nc.sync.dma_start(whh_sb[l][:], whh_d[l].ap())
                nc.sync.dma_start(bias_sb[l][:], bias_d[l].ap())
            ones_sb = constp.tile([1, 512], bf16)
            nc.gpsimd.memset(ones_sb[:], 1.0)

            srctabs = {"node": podtab, "pod": nodetab, "svc": svctab}

            # Conv + LSTM share one scope (and one PSUM pool) so the two
            # phases pipeline: LSTM batch-tile j starts once its 4 conv
            # row-tiles have spilled.
            with tc.tile_pool(name="gat", bufs=6) as gatp, \
                 tc.tile_pool(name="ssb", bufs=6) as ssbp, \
                 tc.tile_pool(name="psum", bufs=2, space="PSUM") as psump, \
                 tc.tile_pool(name="aggsb", bufs=3) as aggsbp, \
                 tc.tile_pool(name="x0sb", bufs=3) as x0sbp, \
                 tc.tile_pool(name="x0res", bufs=2) as x0resp, \
                 tc.tile_pool(name="st_h0", bufs=NJ) as ph0, \
                 tc.tile_pool(name="st_c0", bufs=NJ) as pc0, \
                 tc.tile_pool(name="st_h1", bufs=NJ) as ph1, \
                 tc.tile_pool(name="st_c1", bufs=NJ) as pc1, \
                 tc.tile_pool(name="ifo", bufs=4) as ifop, \
                 tc.tile_pool(name="gt", bufs=4) as gtp, \
                 tc.tile_pool(name="tcl", bufs=4) as tclp, \
                 tc.tile_pool(name="tmp1", bufs=2) as t1p, \
                 tc.tile_pool(name="tmp2", bufs=2) as t2p, \
                 tc.tile_pool(name="xin", bufs=6) as xinp, \
                 tc.tile_pool(name="yout", bufs=3) as youtp:

                def conv_group(d, k0, k1, gs, ss, agg):
                    # PE-accumulate chunk group [k0,k1] into one PSUM partial
                    # (m-outer so per-bank psum accumulation groups stay
                    # sequential), then DVE-combine into agg.
                    pp = psump.tile([P, T * H], f32, tag="ps", name=f"pp_{d}_{k0}")
                    for m in range(8):
                        for kk in range(k0, k1 + 1):
                            nc.tensor.matmul(
                                out=pp[:, m * P:(m + 1) * P],
                                lhsT=gs[kk][:, m * P:(m + 1) * P],
                                rhs=ss[kk][:], start=(kk == k0), stop=(kk == k1))
                    if k0 == 0:
                        nc.vector.tensor_copy(agg[:], pp[:, 0:T * F])
                    else:
                        nc.vector.tensor_tensor(out=agg[:], in0=agg[:],
                                                in1=pp[:, 0:T * F], op=ALU.add)

                N_RES = 2  # LSTM batch tiles whose x0 stays SBUF-resident
                x0res = [x0resp.tile([P, T * 512], bf16, tag="x0r", name=f"x0res_{j}")
                         for j in range(N_RES)]

                def conv_tail(d, kind, agg):
                    # linear per timestep + bias (rank-1 matmul) -> PSUM,
                    # then LeakyReLU(0.01) -> fp16 -> spill (or SBUF-resident)
                    hx = psump.tile([P, T * H], f32, tag="ps", name=f"hx_{d}")
                    wt, bt = wt_sb[kind], bt_sb[kind]
                    for t in range(T):
                        pb = 64 * (t % 2)
                        nc.tensor.matmul(
                            out=hx[:, t * H:(t + 1) * H], lhsT=bt[0:1, t * H:(t + 1) * H],
                            rhs=ones_sb[0:1, 0:P], start=True, stop=False)
                        nc.tensor.matmul(
                            out=hx[:, t * H:(t + 1) * H],
                            lhsT=wt[pb:pb + F, t * H:(t + 1) * H],
                            rhs=agg[pb:pb + F, (t // 2) * P:(t // 2 + 1) * P],
                            start=False, stop=True)
                    j = min(d // 4, NJ - 1)
                    rl = P * (d - 4 * j)
                    if j < N_RES:
                        # Prelu writes straight into the resident tile's
                        # [h, t*512 + rl + r] slices; no DRAM round trip
                        dst = x0res[j][:].rearrange("h (t r) -> h t r", t=T)[:, :, rl:rl + P]
                        nc.scalar.activation(
                            dst, hx[:].rearrange("h (t r) -> h t r", t=T), AF.Prelu, alpha=0.01)
                        return
                    x0t = x0sbp.tile([P, T * H], bf16, tag="x0", name=f"x0t_{d}")
                    nc.scalar.activation(x0t[:], hx[:], AF.Prelu, alpha=0.01)
                    # spill as one contiguous per-partition block (4KB runs, no
                    # sub-512B DMA penalty in the DMA-bound conv window); the
                    # strided cost moves to the LSTM-phase load where DMA is idle
                    nc.sync.dma_start(
                        x0p[j][:, rl * T:(rl + P) * T], x0t[:])

                def conv_tile_pod(d):
                    # Pod conv via one-hot matmuls against the SBUF-resident
                    # node table: C[s,r] = sum_e w_e [src=s][dst=r], then
                    # agg.T = nodetab.T @ C. No DRAM gather at all.
                    Kd = K[d]
                    agg = aggsbp.tile([P, T * F], bf16, tag="agg", name=f"agg_{d}")
                    ohs, s0s = [], []
                    for ki in range(Kd):
                        col = int(base[d]) + ki
                        oh = gatp.tile([P, 512], bf16, tag="g", name=f"oh_{d}_{ki}")
                        nc.vector.tensor_scalar(
                            out=oh[:], in0=iota512_sb[:],
                            scalar1=esrcf_sb[:, col:col + 1], scalar2=ew_sb[:, col:col + 1],
                            op0=ALU.is_equal, op1=ALU.mult)
                        s0 = ssbp.tile([P, P], bf16, tag="s", name=f"s0_{d}_{ki}")
                        nc.vector.tensor_scalar(
                            out=s0[:], in0=iota_sb[:],
                            scalar1=edst_sb[:, col:col + 1], scalar2=None,
                            op0=ALU.is_equal)
                        ohs.append(oh)
                        s0s.append(s0)
                    cps = psump.tile([P, T * H], f32, tag="ps", name=f"cps_{d}")
                    for sc in range(4):
                        for ki in range(Kd):
                            nc.tensor.matmul(
                                out=cps[:, sc * P:(sc + 1) * P],
                                lhsT=ohs[ki][:, sc * P:(sc + 1) * P], rhs=s0s[ki][:],
                                start=(ki == 0), stop=(ki == Kd - 1))
                    csb = ssbp.tile([P, 512], bf16, tag="csb", name=f"csb_{d}")
                    nc.vector.tensor_copy(csb[:], cps[:, 0:512])
                    # reuse the same PSUM tile for agg (start=True overwrites);
                    # keeps pod tiles at 2 psum allocations for 2-slot pipelining
                    for m in range(8):
                        for sc in range(4):
                            nc.tensor.matmul(
                                out=cps[:, m * P:(m + 1) * P],
                                lhsT=ntab_sb[sc][:, m * P:(m + 1) * P],
                                rhs=csb[:, sc * P:(sc + 1) * P],
                                start=(sc == 0), stop=(sc == 3))
                    nc.vector.tensor_copy(agg[:], cps[:, 0:T * F])
                    conv_tail(d, "pod", agg)

                def conv_tile(d):
                    kind = tile_kind[d]
                    Kd = K[d]
                    tab = srctabs[kind]
                    agg = aggsbp.tile([P, T * F], bf16, tag="agg", name=f"agg_{d}")
                    gs, ss = [], []
                    for ki in range(Kd):
                        col = int(base[d]) + ki
                        g = gatp.tile([P, T * F], bf16, tag="g", name=f"g_{d}_{ki}")
                        nc.gpsimd.indirect_dma_start(
                            out=g[:], out_offset=None, in_=tab.ap(),
                            in_offset=bass.IndirectOffsetOnAxis(ap=esrc_sb[:, col:col + 1], axis=0))
                        s = ssbp.tile([P, P], bf16, tag="s", name=f"s_{d}_{ki}")
                        nc.vector.tensor_scalar(
                            out=s[:], in0=iota_sb[:],
                            scalar1=edst_sb[:, col:col + 1], scalar2=ew_sb[:, col:col + 1],
                            op0=ALU.is_equal, op1=ALU.mult)
                        gs.append(g)
                        ss.append(s)
                        if ki % 4 == 3 or ki == Kd - 1:
                            conv_group(d, (ki // 4) * 4, ki, gs, ss, agg)
                    conv_tail(d, kind, agg)

                # ---------------- LSTM phase ----------------
                gatesp = psump

                h = [[None] * NJ, [None] * NJ]
                c = [[None] * NJ, [None] * NJ]
                for j, (r0, B) in enumerate(LSTM_TILES):
                    h[0][j] = ph0.tile([P, B], bf16, tag="h0", name=f"h0_{j}")
                    c[0][j] = pc0.tile([P, B], f32, tag="c0", name=f"c0_{j}")
                    h[1][j] = ph1.tile([P, B], bf16, tag="h1", name=f"h1_{j}")
                    c[1][j] = pc1.tile([P, B], f32, tag="c1", name=f"c1_{j}")

                outr = out_d.ap().rearrange("h (t r) -> h t r", t=T)

                def cell(l, j, t, xin_l, B):
                    # gates [i,f,o,2g]; one Sigmoid covers all four chunks
                    # (g weights pre-doubled; tanh(g) = 2*sigmoid(2g)-1)
                    gates = gatesp.tile([P, 4 * B], f32, tag="ps",
                                        name=f"gates_{l}_{j}_{t}")
                    for i in range(4):
                        nc.tensor.matmul(
                            out=gates[:, i * B:(i + 1) * B],
                            lhsT=bias_sb[l][0:1, i * H:(i + 1) * H],
                            rhs=ones_sb[0:1, 0:B], start=True, stop=False)
                        nc.tensor.matmul(
                            out=gates[:, i * B:(i + 1) * B],
                            lhsT=wih_sb[l][:, i * H:(i + 1) * H],
                            rhs=xin_l[:], start=False, stop=(t == 0))
                        if t > 0:
                            nc.tensor.matmul(
                                out=gates[:, i * B:(i + 1) * B],
                                lhsT=whh_sb[l][:, i * H:(i + 1) * H],
                                rhs=h[l][j][:], start=False, stop=True)
                    sg = ifop.tile([P, 4 * B], bf16, tag="ifo", name=f"sg_{l}_{j}_{t}")
                    nc.scalar.activation(sg[:], gates[:], AF.Sigmoid)
                    gt = gtp.tile([P, B], bf16, tag="gt", name=f"gt_{l}_{j}_{t}")
                    nc.vector.tensor_scalar(
                        out=gt[:], in0=sg[:, 3 * B:4 * B], scalar1=2.0, scalar2=-1.0,
                        op0=ALU.mult, op1=ALU.add)
                    if t == 0:
                        nc.vector.tensor_mul(c[l][j][:], sg[:, 0:B], gt[:])
                    else:
                        t1 = t1p.tile([P, B], f32, tag="t1", name=f"t1_{l}_{j}_{t}")
                        nc.vector.tensor_mul(t1[:], sg[:, B:2 * B], c[l][j][:])
                        t2 = t2p.tile([P, B], bf16, tag="t2", name=f"t2_{l}_{j}_{t}")
                        nc.vector.tensor_mul(t2[:], sg[:, 0:B], gt[:])
                        nc.gpsimd.tensor_tensor(c[l][j][:], t1[:], t2[:], op=ALU.add)
                    tcl = tclp.tile([P, B], bf16, tag="tc", name=f"tc_{l}_{j}_{t}")
                    nc.scalar.activation(tcl[:], c[l][j][:], AF.Tanh)
                    nc.vector.tensor_mul(h[l][j][:], sg[:, 2 * B:3 * B], tcl[:])

                def lstm_batch(tj_list):
                    # emit a batch of cell-steps: x loads, then all L0, then all L1+out
                    xs = {}
                    for t, j in tj_list:
                        if j < 2:
                            xs[(t, j)] = x0res[j][:, t * 512:(t + 1) * 512]
                            continue
                        B = LSTM_TILES[j][1]
                        x = xinp.tile([P, B], bf16, tag="x", name=f"x_{t}_{j}")
                        nc.sync.dma_start(
                            x[:].rearrange("h (dl t r) -> h dl t r", t=1, r=P),
                            x0p[j][:].rearrange("h (dl t r) -> h dl t r", t=T, r=P)
                            [:, :, t:t + 1, :])
                        xs[(t, j)] = x
                    for t, j in tj_list:
                        cell(0, j, t, xs[(t, j)], LSTM_TILES[j][1])
                    for t, j in tj_list:
                        r0, B = LSTM_TILES[j]
                        cell(1, j, t, h[0][j], B)
                        y = youtp.tile([P, B], bf16, tag="y", name=f"y_{t}_{j}")
                        nc.vector.tensor_copy(y[:], h[1][j][:])
                        nc.sync.dma_start(
                            outr[:, t:t + 1, r0:r0 + B].rearrange("h t r -> h (t r)"), y[:])

                for d in range(N_TILES):
                    conv_tile(d)
                for t in range(T):
                    lstm_batch([(t, j) for j in range(NJ)])

    nc.compile()
    return nc


# ----------------------------------------------------------------------------
# Entry points
# ----------------------------------------------------------------------------

def _assemble(results):
    # per-core out: (128, T*R_CORE) viewed [h, t*R_CORE + r] -> (r, t, h)
    full = np.empty((N_NODE + N_POD + N_SVC, T, H), dtype=np.float32)
    parts_node, parts_pod, parts_svc = [], [], []
    for cidx, res in enumerate(results):
        o = res["out"].astype(np.float32).reshape(H, T, R_CORE).transpose(2, 1, 0)  # (r, t, h)
        n_node = min(NODE_PC, max(0, N_NODE - cidx * NODE_PC))
        n_svc = min(SVC_PC, max(0, N_SVC - cidx * SVC_PC))
        parts_pod.append(o[0:POD_PC])
        svc0 = POD_TILES * P
        parts_svc.append(o[svc0:svc0 + n_svc])
        node0 = (POD_TILES + SVC_TILES) * P
        parts_node.append(o[node0:node0 + n_node])
    full[0:N_NODE] = np.concatenate(parts_node, axis=0)
    full[N_NODE:N_NODE + N_POD] = np.concatenate(parts_pod, axis=0)
    full[N_NODE + N_POD:] = np.concatenate(parts_svc, axis=0)
    return full


def run(inputs, trace=False):
    from concourse.bass_utils import run_bass_kernel_spmd
    meta, in_maps = _prep(inputs)
    if meta not in _COMPILED:
        _COMPILED[meta] = _build(meta)
    nc = _COMPILED[meta]
    try:
        res = run_bass_kernel_spmd(nc, in_maps, core_ids=list(range(NCORES)), trace=trace)
    except Exception:
        # transient device errors (e.g. NRT_EXEC_UNIT_UNRECOVERABLE) recover
        # on re-execution; retry once before giving up
        res = run_bass_kernel_spmd(nc, in_maps, core_ids=list(range(NCORES)), trace=trace)
    return _assemble(res.results), res


def kernel(**inputs):
    out, _ = run(inputs, trace=False)
    return out
